# revision 20
# baseline (speedup 1.0000x reference)
"""AFT-Full on 8 TRN2 cores, v2: d-split pair sharding + pairwise AllGather.

Core c: batch b = c//2, parity p = c%2. Each core computes K/V/Q
projections for its d-half only (columns [p*512,(p+1)*512) of the
projection output), the AFT mixing for that d-half over ALL t, then the
pair exchanges activation halves (2 x 0.5 MB AllGather over replica
pairs, launched at 25%/50% of the AFT phase so they hide under compute)
and each core runs the full-d output projection for its own t-half
(rows [p*1024,(p+1)*1024)).

The AFT writes its output in a parity-ROTATED t order (own-t chunks
first, pair-t chunks last) into four separate chunk tiles, so every PE
instruction is static; parity enters only through dynamic-offset
gpsimd DMAs (ew staging source columns, AllGather-output row select)
and dynamic sigQT read offsets on the DVE evacuation ops.

Same bias identities as v1: bk cancels in numer/denom, bv is a
per-partition post-ratio add, bq fuses into the sigmoid, bo rides the
output-projection PSUM evacuation.
"""

import sys

if "/opt/trn_rl_repo" not in sys.path:
    sys.path.insert(0, "/opt/trn_rl_repo")

import numpy as np
import ml_dtypes

BF16 = ml_dtypes.bfloat16

B, T, D = 4, 2048, 1024
TH = T // 2   # own-t rows per core
DH = D // 2   # d-half
P = 128
CH = 512
KT = D // P    # 8 k-tiles (full-d contractions)
DHT = DH // P  # 4 d-tiles in my half
ST = T // P    # 16 s-tiles
TC = T // CH   # 4 t-chunks of the full sequence

_cache = {}


def _build_nc():
    import concourse.mybir as mybir
    import concourse.tile as tile
    from concourse import bacc
    from concourse.bass import ds

    dt = mybir.dt
    BF = dt.bfloat16
    F32 = dt.float32
    Act = mybir.ActivationFunctionType
    Alu = mybir.AluOpType
    PAIRS = [[0, 1], [2, 3], [4, 5], [6, 7]]

    nc = bacc.Bacc("TRN2")

    xT = nc.dram_tensor("xT", [D, T], BF, kind="ExternalInput")
    wqT = nc.dram_tensor("wqT", [D, DH], BF, kind="ExternalInput")
    wkT = nc.dram_tensor("wkT", [D, DH], BF, kind="ExternalInput")
    wvT = nc.dram_tensor("wvT", [D, DH], BF, kind="ExternalInput")
    woT = nc.dram_tensor("woT", [D, D], BF, kind="ExternalInput")
    ewT = nc.dram_tensor("ewT", [T, T], BF, kind="ExternalInput")
    bqc = nc.dram_tensor("bqc", [P, DHT], F32, kind="ExternalInput")
    bvc = nc.dram_tensor("bvc", [P, DHT], F32, kind="ExternalInput")
    bob = nc.dram_tensor("bob", [P, D], F32, kind="ExternalInput")
    y = nc.dram_tensor("y", [TH, D], F32, kind="ExternalOutput")

    xT_v = xT.rearrange("(o p) t -> p o t", p=P)
    wq_v = wqT.rearrange("(o p) e -> p o e", p=P)
    wk_v = wkT.rearrange("(o p) e -> p o e", p=P)
    wv_v = wvT.rearrange("(o p) e -> p o e", p=P)
    wo_v = woT.rearrange("(o p) e -> p o e", p=P)
    ew_v = ewT.rearrange("(o p) t -> p o t", p=P)
    y_v = y.rearrange("(o p) e -> p o e", p=P)

    with tile.TileContext(nc) as tc:
        with (
            tc.tile_pool(name="big", bufs=1) as big,
            tc.tile_pool(name="w", bufs=2) as wpool,
            tc.tile_pool(name="tmp", bufs=3) as tmp,
            tc.tile_pool(name="bias", bufs=1) as biasp,
            tc.tile_pool(name="ew", bufs=2) as ewpool,
            tc.tile_pool(name="sg", bufs=1) as sgpool,
            tc.tile_pool(name="wop", bufs=1) as wopool,
            tc.tile_pool(name="dram", bufs=4, space="DRAM") as dram,
            tc.tile_pool(name="psum", bufs=8, space="PSUM") as psum,
        ):
            pid = nc.partition_id()
            par = pid % 2

            eK = big.tile([P, ST, DH], BF, tag="eK")
            eKV = big.tile([P, ST, DH], BF, tag="eKV")
            sigQT = big.tile([P, DHT, T], BF, tag="sigQT")
            # AFT output in rotated-t order: chunk j holds t columns
            # (j*512 + p*1024) mod 2048 .. +512; j=0,1 own-t, j=2,3 pair-t
            oPC = [
                big.tile([P, DHT, CH], BF, tag=f"oPC{j}", name=f"oPC{j}")
                for j in range(TC)
            ]

            with tc.tile_pool(name="x", bufs=1) as xpool:
                xs = xpool.tile([P, KT, T], BF, tag="xs")
                wk_s = wpool.tile([P, KT, DH], BF, tag="w")
                wv_s = wpool.tile([P, KT, DH], BF, tag="w")

                # PE warm-up during the input-DMA wait
                warm = biasp.tile([P, CH], BF, tag="warm")
                nc.vector.memset(warm[:], 0.0)
                pwarm = psum.tile([P, CH], F32, tag="ps", name="pwarm")
                for _ in range(6):
                    nc.tensor.matmul(
                        pwarm[:], warm[:, :P], warm[:], start=True, stop=True
                    )

                nc.sync.dma_start(xs[:, 0, :P], xT_v[:, 0, :P])
                nc.sync.dma_start(wk_s[:, 0, :], wk_v[:, 0, :])
                nc.sync.dma_start(xs[:, 0, P:], xT_v[:, 0, P:])
                for k in range(1, KT):
                    nc.sync.dma_start(xs[:, k, :], xT_v[:, k, :])
                    nc.sync.dma_start(wk_s[:, k, :], wk_v[:, k, :])
                for k in range(KT):
                    nc.sync.dma_start(wv_s[:, k, :], wv_v[:, k, :])
                wq_s = wpool.tile([P, KT, DH], BF, tag="w")
                for k in range(KT):
                    nc.sync.dma_start(wq_s[:, k, :], wq_v[:, k, :])
                bq_s = biasp.tile([P, DHT], F32, tag="bq")
                nc.sync.dma_start(bq_s[:], bqc[:])
                bv_s = biasp.tile([P, DHT], F32, tag="bv")
                nc.sync.dma_start(bv_s[:], bvc[:])
                bo_s = biasp.tile([P, D], F32, tag="bo")
                nc.sync.dma_start(bo_s[:], bob[:])

                # tiny gpsimd read of the last wq slice: delays the (gpsimd)
                # ewT chunk DMAs below until the input stream has landed, so
                # they don't steal HBM bandwidth during the compute ramp
                dep = biasp.tile([1, 2], BF, tag="dep")
                nc.gpsimd.dma_start(dep[:], wq_s[:1, KT - 1, :2])

                # ---- K projection (d-half) -> eK, k-outer in 2 groups ----
                for g in range(2):
                    sts = list(range(g * 8, g * 8 + 8))
                    pks = {
                        st: psum.tile([P, CH], F32, tag="ps", name=f"pk{st}")
                        for st in sts
                    }
                    for k in range(KT):
                        for st in sts:
                            nc.tensor.matmul(
                                pks[st][:],
                                xs[:, k, st * P : (st + 1) * P],
                                wk_s[:, k, :],
                                start=(k == 0), stop=(k == KT - 1),
                            )
                    for st in sts:
                        nc.scalar.activation(eK[:, st, :], pks[st][:], Act.Exp)

                # ---- V projection (d-half) -> eKV = eK * V ----
                for st in range(ST):
                    pv = psum.tile([P, CH], F32, tag="ps")
                    for k in range(KT):
                        nc.tensor.matmul(
                            pv[:],
                            xs[:, k, st * P : (st + 1) * P],
                            wv_s[:, k, :],
                            start=(k == 0), stop=(k == KT - 1),
                        )
                    nc.vector.tensor_tensor(
                        eKV[:, st, :], eK[:, st, :], pv[:], Alu.mult
                    )

                # ---- Q^T projection (d-half e, ALL t) ----
                for et in range(DHT):
                    esl = slice(et * P, (et + 1) * P)
                    for c in range(TC):
                        tsl = slice(c * CH, (c + 1) * CH)
                        pq = psum.tile([P, CH], F32, tag="ps")
                        for k in range(KT):
                            nc.tensor.matmul(
                                pq[:], wq_s[:, k, esl], xs[:, k, tsl],
                                start=(k == 0), stop=(k == KT - 1),
                            )
                        nc.scalar.activation(
                            sigQT[:, et, tsl], pq[:], Act.Sigmoid,
                            bias=bq_s[:, et : et + 1],
                        )

            # ---- AFT over rotated t-chunks; j = 2,3 (pair-t) first so the
            # pairwise exchanges launch early and hide under compute ----
            b_in = [None, None]
            S_out = [None, None]
            S_sb = [None, None]
            for i in range(2):
                b_in[i] = dram.tile([DH, CH], BF, name=f"bin{i}")
                S_out[i] = dram.tile([2 * DH, CH], BF, name=f"sout{i}")
                S_sb[i] = sgpool.tile(
                    [P, DHT, CH], BF, tag=f"ssb{i}", name=f"ssb{i}"
                )

            # All (dynamic, gpsimd) ew staging DMAs are emitted BEFORE the
            # AllGather triggers: the collective's completion wait blocks
            # the gpsimd FIFO, so anything gpsimd-issued after it would
            # stall behind the exchange.
            for pos, j in enumerate([2, 3, 0, 1]):
                # ew^T chunk for rotated chunk j: t columns
                # (j*512 + p*1024) & 2047
                toff = nc.s_assert_within(
                    (j * CH + par * TH) & (T - 1), 0, T - CH,
                    skip_runtime_assert=True,
                )
                ewc = ewpool.tile([P, ST, CH], BF, tag="ewc")
                nc.gpsimd.dma_start(
                    ewc[:, : ST // 2, :], ew_v[:, : ST // 2, ds(toff, CH)]
                )
                nc.gpsimd.dma_start(
                    ewc[:, ST // 2 :, :], ew_v[:, ST // 2 :, ds(toff, CH)]
                )

                for dti in range(DHT):
                    dsl = slice(dti * P, (dti + 1) * P)
                    pn = psum.tile([P, CH], F32, tag="ps")
                    pd = psum.tile([P, CH], F32, tag="ps")
                    for ss in range(ST):
                        nc.tensor.matmul(
                            pn[:], eKV[:, ss, dsl], ewc[:, ss, :],
                            start=(ss == 0), stop=(ss == ST - 1),
                        )
                    for ss in range(ST):
                        nc.tensor.matmul(
                            pd[:], eK[:, ss, dsl], ewc[:, ss, :],
                            start=(ss == 0), stop=(ss == ST - 1),
                        )
                    rec = tmp.tile([P, CH], F32, tag="rec")
                    nc.vector.reciprocal_approx_fast(rec[:], pd[:])
                    rat = tmp.tile([P, CH], F32, tag="rat")
                    nc.vector.tensor_tensor(rat[:], pn[:], rec[:], Alu.mult)
                    nc.vector.tensor_scalar(
                        rat[:], rat[:], bv_s[:, dti : dti + 1], None, Alu.add
                    )
                    nc.vector.tensor_tensor(
                        oPC[j][:, dti, :], rat[:],
                        sigQT[:, dti, ds(toff, CH)], Alu.mult,
                    )

                if pos < 2:
                    # ship pair-t chunk to the DRAM bounce buffer (sync
                    # engine, so it fires as soon as the chunk is done)
                    nc.sync.dma_start(
                        b_in[pos].rearrange("(o p) t -> p o t", p=P)[:],
                        oPC[j][:],
                    )

            for i in range(2):
                nc.gpsimd.collective_compute(
                    "AllGather",
                    Alu.bypass,
                    replica_groups=PAIRS,
                    ins=[b_in[i].opt()],
                    outs=[S_out[i].opt()],
                )
                # pull the PAIR's contribution rows (their d-half for my
                # own-t chunk i): rank-row select is parity-dependent
                nc.gpsimd.dma_start(
                    S_sb[i][:],
                    S_out[i].rearrange("(o p) t -> p o t", p=P)[
                        :, ds((1 - par) * DHT, DHT), :
                    ],
                )

            # ---- output projection for own-t rows, full d ----
            # contraction order: k 0..3 = my d-half (oPC[0], oPC[1]),
            # k 4..7 = pair d-half (S_sb); woT rows are host-permuted to
            # [my half; pair half].
            wo_s = wopool.tile([P, KT, D], BF, tag="wo", name="wo_s")
            for k in range(KT):
                nc.sync.dma_start(wo_s[:, k, :], wo_v[:, k, :])
            for tt in range(TH // P):
                own = oPC[tt // DHT]        # chunk 0 or 1 (own-t)
                ssb = S_sb[tt // DHT]
                tof = (tt % DHT) * P
                for ec in range(2):
                    esl = slice(ec * CH, (ec + 1) * CH)
                    py = psum.tile([P, CH], F32, tag="ps")
                    for k in range(4):
                        nc.tensor.matmul(
                            py[:], own[:, k, tof : tof + P], wo_s[:, k, esl],
                            start=(k == 0), stop=False,
                        )
                    for k in range(4):
                        nc.tensor.matmul(
                            py[:], ssb[:, k, tof : tof + P],
                            wo_s[:, 4 + k, esl],
                            start=False, stop=(k == 3),
                        )
                    ysb = tmp.tile([P, CH], F32, tag="ysb")
                    nc.vector.tensor_tensor(ysb[:], py[:], bo_s[:, esl], Alu.add)
                    nc.sync.dma_start(y_v[:, tt, esl], ysb[:])

    nc.compile()
    return nc


def _get_nc():
    if "nc" not in _cache:
        _cache["nc"] = _build_nc()
    return _cache["nc"]


def kernel(x, dummy, Wq, bq, Wk, bk, Wv, bv, Wo, bo, wbias):
    import os

    x = np.asarray(x, np.float32)
    Wq = np.asarray(Wq, np.float32)
    Wk = np.asarray(Wk, np.float32)
    Wv = np.asarray(Wv, np.float32)
    Wo = np.asarray(Wo, np.float32)
    bq = np.asarray(bq, np.float32)
    bv = np.asarray(bv, np.float32)
    bo = np.asarray(bo, np.float32)
    wbias = np.asarray(wbias, np.float32)

    wqTf = np.ascontiguousarray(Wq.T)  # [d_in, e_out] fp32
    wkTf = np.ascontiguousarray(Wk.T)
    wvTf = np.ascontiguousarray(Wv.T)
    woTf = np.ascontiguousarray(Wo.T)  # rows = d
    ewTb = np.ascontiguousarray(np.exp(wbias).T).astype(BF16)  # [s, t]
    bob = np.ascontiguousarray(np.broadcast_to(bo, (P, D)))

    in_maps = []
    for c in range(8):
        b, p = c // 2, c % 2
        dlo, dhi = p * DH, (p + 1) * DH
        qlo, qhi = (1 - p) * DH, (2 - p) * DH
        woTp = np.concatenate([woTf[dlo:dhi], woTf[qlo:qhi]], axis=0)
        in_maps.append(
            {
                "xT": np.ascontiguousarray(x[b].T).astype(BF16),
                "wqT": np.ascontiguousarray(wqTf[:, dlo:dhi]).astype(BF16),
                "wkT": np.ascontiguousarray(wkTf[:, dlo:dhi]).astype(BF16),
                "wvT": np.ascontiguousarray(wvTf[:, dlo:dhi]).astype(BF16),
                "woT": np.ascontiguousarray(woTp).astype(BF16),
                "ewT": ewTb,
                "bqc": np.ascontiguousarray(bq[dlo:dhi].reshape(DHT, P).T),
                "bvc": np.ascontiguousarray(bv[dlo:dhi].reshape(DHT, P).T),
                "bob": bob,
            }
        )

    from concourse.bass_utils import run_bass_kernel_spmd

    nc = _get_nc()
    trace = bool(os.environ.get("AFT_TRACE"))
    if not trace:
        # keep run_bass_kernel_spmd off the (environment-dependent) NTFF
        # profiling path unless explicitly requested
        os.environ["BASS_NEVER_TRACE"] = "1"
    res = run_bass_kernel_spmd(
        nc, in_maps, core_ids=list(range(8)), trace=trace
    )
    kernel._last_exec_ns = res.exec_time_ns
    kernel._last_result = res

    out = np.empty((B, T, D), np.float32)
    for c in range(8):
        b, p = c // 2, c % 2
        out[b, p * TH : (p + 1) * TH, :] = res.results[c]["y"]
    return out


# revision 26
# speedup vs baseline: 1.0171x; 1.0171x over previous
"""AFT-Full on 8 TRN2 cores, v2: d-split pair sharding + pairwise AllGather.

Core c: batch b = c//2, parity p = c%2. Each core computes K/V/Q
projections for its d-half only (columns [p*512,(p+1)*512) of the
projection output), the AFT mixing for that d-half over ALL t, then the
pair exchanges activation halves (2 x 0.5 MB AllGather over replica
pairs, launched at 25%/50% of the AFT phase so they hide under compute)
and each core runs the full-d output projection for its own t-half
(rows [p*1024,(p+1)*1024)).

The AFT writes its output in a parity-ROTATED t order (own-t chunks
first, pair-t chunks last) into four separate chunk tiles, so every PE
instruction is static; parity enters only through dynamic-offset
gpsimd DMAs (ew staging source columns, AllGather-output row select)
and dynamic sigQT read offsets on the DVE evacuation ops.

Same bias identities as v1: bk cancels in numer/denom, bv is a
per-partition post-ratio add, bq fuses into the sigmoid, bo rides the
output-projection PSUM evacuation.
"""

import sys

if "/opt/trn_rl_repo" not in sys.path:
    sys.path.insert(0, "/opt/trn_rl_repo")

import numpy as np
import ml_dtypes

BF16 = ml_dtypes.bfloat16

B, T, D = 4, 2048, 1024
TH = T // 2   # own-t rows per core
DH = D // 2   # d-half
P = 128
CH = 512
KT = D // P    # 8 k-tiles (full-d contractions)
DHT = DH // P  # 4 d-tiles in my half
ST = T // P    # 16 s-tiles
TC = T // CH   # 4 t-chunks of the full sequence

_cache = {}


def _build_nc():
    import concourse.mybir as mybir
    import concourse.tile as tile
    from concourse import bacc
    from concourse.bass import ds

    dt = mybir.dt
    BF = dt.bfloat16
    F32 = dt.float32
    Act = mybir.ActivationFunctionType
    Alu = mybir.AluOpType
    PAIRS = [[0, 1], [2, 3], [4, 5], [6, 7]]

    nc = bacc.Bacc("TRN2")

    xT = nc.dram_tensor("xT", [D, T], BF, kind="ExternalInput")
    wqT = nc.dram_tensor("wqT", [D, DH], BF, kind="ExternalInput")
    wkT = nc.dram_tensor("wkT", [D, DH], BF, kind="ExternalInput")
    wvT = nc.dram_tensor("wvT", [D, DH], BF, kind="ExternalInput")
    woT = nc.dram_tensor("woT", [D, D], BF, kind="ExternalInput")
    ewT = nc.dram_tensor("ewT", [T, T], BF, kind="ExternalInput")
    bqc = nc.dram_tensor("bqc", [P, DHT], F32, kind="ExternalInput")
    bvc = nc.dram_tensor("bvc", [P, DHT], F32, kind="ExternalInput")
    bob = nc.dram_tensor("bob", [P, D], F32, kind="ExternalInput")
    y = nc.dram_tensor("y", [TH, D], F32, kind="ExternalOutput")

    xT_v = xT.rearrange("(o p) t -> p o t", p=P)
    wq_v = wqT.rearrange("(o p) e -> p o e", p=P)
    wk_v = wkT.rearrange("(o p) e -> p o e", p=P)
    wv_v = wvT.rearrange("(o p) e -> p o e", p=P)
    wo_v = woT.rearrange("(o p) e -> p o e", p=P)
    ew_v = ewT.rearrange("(o p) t -> p o t", p=P)
    y_v = y.rearrange("(o p) e -> p o e", p=P)

    with tile.TileContext(nc) as tc:
        with (
            tc.tile_pool(name="big", bufs=1) as big,
            tc.tile_pool(name="w", bufs=2) as wpool,
            tc.tile_pool(name="tmp", bufs=3) as tmp,
            tc.tile_pool(name="bias", bufs=1) as biasp,
            tc.tile_pool(name="ew", bufs=2) as ewpool,
            tc.tile_pool(name="sg", bufs=1) as sgpool,
            tc.tile_pool(name="wop", bufs=1) as wopool,
            tc.tile_pool(name="dram", bufs=4, space="DRAM") as dram,
            tc.tile_pool(name="psum", bufs=8, space="PSUM") as psum,
        ):
            pid = nc.partition_id()
            par = pid % 2

            eK = big.tile([P, ST, DH], BF, tag="eK")
            eKV = big.tile([P, ST, DH], BF, tag="eKV")
            sigQT = big.tile([P, DHT, T], BF, tag="sigQT")
            # AFT output in rotated-t order: chunk j holds t columns
            # (j*512 + p*1024) mod 2048 .. +512; j=0,1 own-t, j=2,3 pair-t
            oPC = [
                big.tile([P, DHT, CH], BF, tag=f"oPC{j}", name=f"oPC{j}")
                for j in range(TC)
            ]

            with tc.tile_pool(name="x", bufs=1) as xpool:
                xs = xpool.tile([P, KT, T], BF, tag="xs")
                wk_s = wpool.tile([P, KT, DH], BF, tag="w")
                wv_s = wpool.tile([P, KT, DH], BF, tag="w")

                # PE warm-up during the input-DMA wait
                warm = biasp.tile([P, CH], BF, tag="warm")
                nc.vector.memset(warm[:], 0.0)
                pwarm = psum.tile([P, CH], F32, tag="ps", name="pwarm")
                for _ in range(6):
                    nc.tensor.matmul(
                        pwarm[:], warm[:, :P], warm[:], start=True, stop=True
                    )

                # Only the K-projection's inputs (xs + wk, 5 MB) go on the
                # parallel HW DMA queues: all 16 queues drain concurrently,
                # so anything else emitted here would steal ramp bandwidth
                # from the critical stream.
                for k in range(KT):
                    nc.sync.dma_start(wk_s[:, k, :], wk_v[:, k, :])
                for k in range(KT):
                    nc.sync.dma_start(xs[:, k, :TH], xT_v[:, k, :TH])
                bq_s = biasp.tile([P, DHT], F32, tag="bq")
                nc.sync.dma_start(bq_s[:], bqc[:])
                bv_s = biasp.tile([P, DHT], F32, tag="bv")
                nc.sync.dma_start(bv_s[:], bvc[:])

                # Later-needed inputs ride the gpsimd FIFO behind a tiny
                # dependency gate on the last critical slice: the HW queues
                # drain everything enqueued in parallel, so only the 3 MB
                # the K projection's first group needs goes on them above.
                dep = biasp.tile([1, 32], BF, tag="dep")
                nc.gpsimd.dma_start(dep[:], xs[:1, KT - 1, TH - 32 : TH])
                for k in range(KT):
                    nc.gpsimd.dma_start(xs[:, k, TH:], xT_v[:, k, TH:])
                for k in range(KT):
                    nc.gpsimd.dma_start(wv_s[:, k, :], wv_v[:, k, :])
                wq_s = wpool.tile([P, KT, DH], BF, tag="w")
                for k in range(KT):
                    nc.gpsimd.dma_start(wq_s[:, k, :], wq_v[:, k, :])
                bo_s = biasp.tile([P, D], F32, tag="bo")
                nc.gpsimd.dma_start(bo_s[:], bob[:])

                # ---- K projection (d-half) -> eK, k-outer in 2 groups ----
                for g in range(2):
                    sts = list(range(g * 8, g * 8 + 8))
                    pks = {
                        st: psum.tile([P, CH], F32, tag="ps", name=f"pk{st}")
                        for st in sts
                    }
                    for k in range(KT):
                        for st in sts:
                            nc.tensor.matmul(
                                pks[st][:],
                                xs[:, k, st * P : (st + 1) * P],
                                wk_s[:, k, :],
                                start=(k == 0), stop=(k == KT - 1),
                            )
                    for st in sts:
                        nc.scalar.activation(eK[:, st, :], pks[st][:], Act.Exp)

                # ---- V projection (d-half) -> eKV = eK * V ----
                # k-outer like K: wv slices arrive staggered off the gated
                # gpsimd chain, so consume them in arrival order
                for g in range(2):
                    sts = list(range(g * 8, g * 8 + 8))
                    pvs = {
                        st: psum.tile([P, CH], F32, tag="ps", name=f"pv{st}")
                        for st in sts
                    }
                    for k in range(KT):
                        for st in sts:
                            nc.tensor.matmul(
                                pvs[st][:],
                                xs[:, k, st * P : (st + 1) * P],
                                wv_s[:, k, :],
                                start=(k == 0), stop=(k == KT - 1),
                            )
                    for st in sts:
                        nc.vector.tensor_tensor(
                            eKV[:, st, :], eK[:, st, :], pvs[st][:], Alu.mult
                        )

                # ---- Q^T projection (d-half e, ALL t) ----
                for et in range(DHT):
                    esl = slice(et * P, (et + 1) * P)
                    for c in range(TC):
                        tsl = slice(c * CH, (c + 1) * CH)
                        pq = psum.tile([P, CH], F32, tag="ps")
                        for k in range(KT):
                            nc.tensor.matmul(
                                pq[:], wq_s[:, k, esl], xs[:, k, tsl],
                                start=(k == 0), stop=(k == KT - 1),
                            )
                        nc.scalar.activation(
                            sigQT[:, et, tsl], pq[:], Act.Sigmoid,
                            bias=bq_s[:, et : et + 1],
                        )

            # ---- AFT over rotated t-chunks; j = 2,3 (pair-t) first so the
            # pairwise exchanges launch early and hide under compute ----
            b_in = [None, None]
            S_out = [None, None]
            S_sb = [None, None]
            for i in range(2):
                b_in[i] = dram.tile([DH, CH], BF, name=f"bin{i}")
                S_out[i] = dram.tile([2 * DH, CH], BF, name=f"sout{i}")
                S_sb[i] = sgpool.tile(
                    [P, DHT, CH], BF, tag=f"ssb{i}", name=f"ssb{i}"
                )

            # All (dynamic, gpsimd) ew staging DMAs are emitted BEFORE the
            # AllGather triggers: the collective's completion wait blocks
            # the gpsimd FIFO, so anything gpsimd-issued after it would
            # stall behind the exchange.
            for pos, j in enumerate([2, 3, 0, 1]):
                # ew^T chunk for rotated chunk j: t columns
                # (j*512 + p*1024) & 2047
                toff = nc.s_assert_within(
                    (j * CH + par * TH) & (T - 1), 0, T - CH,
                    skip_runtime_assert=True,
                )
                ewc = ewpool.tile([P, ST, CH], BF, tag="ewc")
                nc.gpsimd.dma_start(
                    ewc[:, : ST // 2, :], ew_v[:, : ST // 2, ds(toff, CH)]
                )
                nc.gpsimd.dma_start(
                    ewc[:, ST // 2 :, :], ew_v[:, ST // 2 :, ds(toff, CH)]
                )

                for dti in range(DHT):
                    dsl = slice(dti * P, (dti + 1) * P)
                    pn = psum.tile([P, CH], F32, tag="ps")
                    pd = psum.tile([P, CH], F32, tag="ps")
                    for ss in range(ST):
                        nc.tensor.matmul(
                            pn[:], eKV[:, ss, dsl], ewc[:, ss, :],
                            start=(ss == 0), stop=(ss == ST - 1),
                        )
                    for ss in range(ST):
                        nc.tensor.matmul(
                            pd[:], eK[:, ss, dsl], ewc[:, ss, :],
                            start=(ss == 0), stop=(ss == ST - 1),
                        )
                    rec = tmp.tile([P, CH], F32, tag="rec")
                    nc.vector.reciprocal_approx_fast(rec[:], pd[:])
                    rat = tmp.tile([P, CH], F32, tag="rat")
                    nc.vector.tensor_tensor(rat[:], pn[:], rec[:], Alu.mult)
                    nc.vector.tensor_scalar(
                        rat[:], rat[:], bv_s[:, dti : dti + 1], None, Alu.add
                    )
                    nc.vector.tensor_tensor(
                        oPC[j][:, dti, :], rat[:],
                        sigQT[:, dti, ds(toff, CH)], Alu.mult,
                    )

                if pos < 2:
                    # ship pair-t chunk to the DRAM bounce buffer (sync
                    # engine, so it fires as soon as the chunk is done)
                    nc.sync.dma_start(
                        b_in[pos].rearrange("(o p) t -> p o t", p=P)[:],
                        oPC[j][:],
                    )

            for i in range(2):
                nc.gpsimd.collective_compute(
                    "AllGather",
                    Alu.bypass,
                    replica_groups=PAIRS,
                    ins=[b_in[i].opt()],
                    outs=[S_out[i].opt()],
                )
                # pull the PAIR's contribution rows (their d-half for my
                # own-t chunk i): rank-row select is parity-dependent
                nc.gpsimd.dma_start(
                    S_sb[i][:],
                    S_out[i].rearrange("(o p) t -> p o t", p=P)[
                        :, ds((1 - par) * DHT, DHT), :
                    ],
                )

            # ---- output projection for own-t rows, full d ----
            # contraction order: k 0..3 = my d-half (oPC[0], oPC[1]),
            # k 4..7 = pair d-half (S_sb); woT rows are host-permuted to
            # [my half; pair half].
            wo_s = wopool.tile([P, KT, D], BF, tag="wo", name="wo_s")
            for k in range(KT):
                nc.sync.dma_start(wo_s[:, k, :], wo_v[:, k, :])
            for tt in range(TH // P):
                own = oPC[tt // DHT]        # chunk 0 or 1 (own-t)
                ssb = S_sb[tt // DHT]
                tof = (tt % DHT) * P
                for ec in range(2):
                    esl = slice(ec * CH, (ec + 1) * CH)
                    py = psum.tile([P, CH], F32, tag="ps")
                    for k in range(4):
                        nc.tensor.matmul(
                            py[:], own[:, k, tof : tof + P], wo_s[:, k, esl],
                            start=(k == 0), stop=False,
                        )
                    for k in range(4):
                        nc.tensor.matmul(
                            py[:], ssb[:, k, tof : tof + P],
                            wo_s[:, 4 + k, esl],
                            start=False, stop=(k == 3),
                        )
                    ysb = tmp.tile([P, CH], F32, tag="ysb")
                    nc.vector.tensor_tensor(ysb[:], py[:], bo_s[:, esl], Alu.add)
                    nc.sync.dma_start(y_v[:, tt, esl], ysb[:])

    nc.compile()
    return nc


def _get_nc():
    if "nc" not in _cache:
        _cache["nc"] = _build_nc()
    return _cache["nc"]


def kernel(x, dummy, Wq, bq, Wk, bk, Wv, bv, Wo, bo, wbias):
    import os

    x = np.asarray(x, np.float32)
    Wq = np.asarray(Wq, np.float32)
    Wk = np.asarray(Wk, np.float32)
    Wv = np.asarray(Wv, np.float32)
    Wo = np.asarray(Wo, np.float32)
    bq = np.asarray(bq, np.float32)
    bv = np.asarray(bv, np.float32)
    bo = np.asarray(bo, np.float32)
    wbias = np.asarray(wbias, np.float32)

    wqTf = np.ascontiguousarray(Wq.T)  # [d_in, e_out] fp32
    wkTf = np.ascontiguousarray(Wk.T)
    wvTf = np.ascontiguousarray(Wv.T)
    woTf = np.ascontiguousarray(Wo.T)  # rows = d
    ewTb = np.ascontiguousarray(np.exp(wbias).T).astype(BF16)  # [s, t]
    bob = np.ascontiguousarray(np.broadcast_to(bo, (P, D)))

    in_maps = []
    for c in range(8):
        b, p = c // 2, c % 2
        dlo, dhi = p * DH, (p + 1) * DH
        qlo, qhi = (1 - p) * DH, (2 - p) * DH
        woTp = np.concatenate([woTf[dlo:dhi], woTf[qlo:qhi]], axis=0)
        in_maps.append(
            {
                "xT": np.ascontiguousarray(x[b].T).astype(BF16),
                "wqT": np.ascontiguousarray(wqTf[:, dlo:dhi]).astype(BF16),
                "wkT": np.ascontiguousarray(wkTf[:, dlo:dhi]).astype(BF16),
                "wvT": np.ascontiguousarray(wvTf[:, dlo:dhi]).astype(BF16),
                "woT": np.ascontiguousarray(woTp).astype(BF16),
                "ewT": ewTb,
                "bqc": np.ascontiguousarray(bq[dlo:dhi].reshape(DHT, P).T),
                "bvc": np.ascontiguousarray(bv[dlo:dhi].reshape(DHT, P).T),
                "bob": bob,
            }
        )

    from concourse.bass_utils import run_bass_kernel_spmd

    nc = _get_nc()
    trace = bool(os.environ.get("AFT_TRACE"))
    if not trace:
        # keep run_bass_kernel_spmd off the (environment-dependent) NTFF
        # profiling path unless explicitly requested
        os.environ["BASS_NEVER_TRACE"] = "1"
    res = run_bass_kernel_spmd(
        nc, in_maps, core_ids=list(range(8)), trace=trace
    )
    kernel._last_exec_ns = res.exec_time_ns
    kernel._last_result = res

    out = np.empty((B, T, D), np.float32)
    for c in range(8):
        b, p = c // 2, c % 2
        out[b, p * TH : (p + 1) * TH, :] = res.results[c]["y"]
    return out


# revision 27
# speedup vs baseline: 1.0330x; 1.0156x over previous
"""AFT-Full on 8 TRN2 cores, v2: d-split pair sharding + pairwise AllGather.

Core c: batch b = c//2, parity p = c%2. Each core computes K/V/Q
projections for its d-half only (columns [p*512,(p+1)*512) of the
projection output), the AFT mixing for that d-half over ALL t, then the
pair exchanges activation halves (2 x 0.5 MB AllGather over replica
pairs, launched at 25%/50% of the AFT phase so they hide under compute)
and each core runs the full-d output projection for its own t-half
(rows [p*1024,(p+1)*1024)).

The AFT writes its output in a parity-ROTATED t order (own-t chunks
first, pair-t chunks last) into four separate chunk tiles, so every PE
instruction is static; parity enters only through dynamic-offset
gpsimd DMAs (ew staging source columns, AllGather-output row select)
and dynamic sigQT read offsets on the DVE evacuation ops.

Same bias identities as v1: bk cancels in numer/denom, bv is a
per-partition post-ratio add, bq fuses into the sigmoid, bo rides the
output-projection PSUM evacuation.
"""

import sys

if "/opt/trn_rl_repo" not in sys.path:
    sys.path.insert(0, "/opt/trn_rl_repo")

import numpy as np
import ml_dtypes

BF16 = ml_dtypes.bfloat16

B, T, D = 4, 2048, 1024
TH = T // 2   # own-t rows per core
DH = D // 2   # d-half
P = 128
CH = 512
KT = D // P    # 8 k-tiles (full-d contractions)
DHT = DH // P  # 4 d-tiles in my half
ST = T // P    # 16 s-tiles
TC = T // CH   # 4 t-chunks of the full sequence

_cache = {}


def _build_nc():
    import concourse.mybir as mybir
    import concourse.tile as tile
    from concourse import bacc
    from concourse.bass import ds

    dt = mybir.dt
    BF = dt.bfloat16
    F32 = dt.float32
    Act = mybir.ActivationFunctionType
    Alu = mybir.AluOpType
    PAIRS = [[0, 1], [2, 3], [4, 5], [6, 7]]

    nc = bacc.Bacc("TRN2")

    xT = nc.dram_tensor("xT", [D, T], BF, kind="ExternalInput")
    wqT = nc.dram_tensor("wqT", [D, DH], BF, kind="ExternalInput")
    wkT = nc.dram_tensor("wkT", [D, DH], BF, kind="ExternalInput")
    wvT = nc.dram_tensor("wvT", [D, DH], BF, kind="ExternalInput")
    woT = nc.dram_tensor("woT", [D, D], BF, kind="ExternalInput")
    ewT = nc.dram_tensor("ewT", [T, T], BF, kind="ExternalInput")
    bqc = nc.dram_tensor("bqc", [P, DHT], F32, kind="ExternalInput")
    bvc = nc.dram_tensor("bvc", [P, DHT], F32, kind="ExternalInput")
    bob = nc.dram_tensor("bob", [P, D], F32, kind="ExternalInput")
    y = nc.dram_tensor("y", [TH, D], F32, kind="ExternalOutput")

    xT_v = xT.rearrange("(o p) t -> p o t", p=P)
    wq_v = wqT.rearrange("(o p) e -> p o e", p=P)
    wk_v = wkT.rearrange("(o p) e -> p o e", p=P)
    wv_v = wvT.rearrange("(o p) e -> p o e", p=P)
    wo_v = woT.rearrange("(o p) e -> p o e", p=P)
    ew_v = ewT.rearrange("(o p) t -> p o t", p=P)
    y_v = y.rearrange("(o p) e -> p o e", p=P)

    with tile.TileContext(nc) as tc:
        with (
            tc.tile_pool(name="big", bufs=1) as big,
            tc.tile_pool(name="w", bufs=2) as wpool,
            tc.tile_pool(name="tmp", bufs=3) as tmp,
            tc.tile_pool(name="bias", bufs=1) as biasp,
            tc.tile_pool(name="ew", bufs=2) as ewpool,
            tc.tile_pool(name="sg", bufs=1) as sgpool,
            tc.tile_pool(name="wop", bufs=1) as wopool,
            tc.tile_pool(name="dram", bufs=4, space="DRAM") as dram,
            tc.tile_pool(name="psum", bufs=8, space="PSUM") as psum,
        ):
            pid = nc.partition_id()
            par = pid % 2

            eK = big.tile([P, ST, DH], BF, tag="eK")
            eKV = big.tile([P, ST, DH], BF, tag="eKV")
            sigQT = big.tile([P, DHT, T], BF, tag="sigQT")
            # AFT output in rotated-t order: chunk j holds t columns
            # (j*512 + p*1024) mod 2048 .. +512; j=0,1 own-t, j=2,3 pair-t
            oPC = [
                big.tile([P, DHT, CH], BF, tag=f"oPC{j}", name=f"oPC{j}")
                for j in range(TC)
            ]

            with tc.tile_pool(name="x", bufs=1) as xpool:
                xs = xpool.tile([P, KT, T], BF, tag="xs")
                wk_s = wpool.tile([P, KT, DH], BF, tag="w")
                wv_s = wpool.tile([P, KT, DH], BF, tag="w")

                # PE warm-up during the input-DMA wait
                warm = biasp.tile([P, CH], BF, tag="warm")
                nc.vector.memset(warm[:], 0.0)
                pwarm = psum.tile([P, CH], F32, tag="ps", name="pwarm")
                for _ in range(6):
                    nc.tensor.matmul(
                        pwarm[:], warm[:, :P], warm[:], start=True, stop=True
                    )

                # Only the K-projection's inputs (xs + wk, 5 MB) go on the
                # parallel HW DMA queues: all 16 queues drain concurrently,
                # so anything else emitted here would steal ramp bandwidth
                # from the critical stream.
                # Two HWDGE pools drain independently: the first two
                # k-slices go alone on the sync queue (land ~11 us), the
                # rest of the critical prefix on the scalar-engine queue.
                for k in range(2):
                    nc.sync.dma_start(wk_s[:, k, :], wk_v[:, k, :])
                    nc.sync.dma_start(xs[:, k, :TH], xT_v[:, k, :TH])
                for k in range(2, KT):
                    nc.scalar.dma_start(wk_s[:, k, :], wk_v[:, k, :])
                    nc.scalar.dma_start(xs[:, k, :TH], xT_v[:, k, :TH])
                bq_s = biasp.tile([P, DHT], F32, tag="bq")
                nc.scalar.dma_start(bq_s[:], bqc[:])
                bv_s = biasp.tile([P, DHT], F32, tag="bv")
                nc.scalar.dma_start(bv_s[:], bvc[:])

                # Later-needed inputs ride the gpsimd FIFO behind a tiny
                # dependency gate on the last critical slice: the HW queues
                # drain everything enqueued in parallel, so only the 3 MB
                # the K projection's first group needs goes on them above.
                dep = biasp.tile([1, 32], BF, tag="dep")
                nc.gpsimd.dma_start(dep[:], xs[:1, KT - 1, TH - 32 : TH])
                for k in range(KT):
                    nc.gpsimd.dma_start(xs[:, k, TH:], xT_v[:, k, TH:])
                for k in range(KT):
                    nc.gpsimd.dma_start(wv_s[:, k, :], wv_v[:, k, :])
                wq_s = wpool.tile([P, KT, DH], BF, tag="w")
                for k in range(KT):
                    nc.gpsimd.dma_start(wq_s[:, k, :], wq_v[:, k, :])
                bo_s = biasp.tile([P, D], F32, tag="bo")
                nc.gpsimd.dma_start(bo_s[:], bob[:])

                # ---- K projection (d-half) -> eK, k-outer in 2 groups ----
                for g in range(2):
                    sts = list(range(g * 8, g * 8 + 8))
                    pks = {
                        st: psum.tile([P, CH], F32, tag="ps", name=f"pk{st}")
                        for st in sts
                    }
                    for k in range(KT):
                        for st in sts:
                            nc.tensor.matmul(
                                pks[st][:],
                                xs[:, k, st * P : (st + 1) * P],
                                wk_s[:, k, :],
                                start=(k == 0), stop=(k == KT - 1),
                            )
                    for st in sts:
                        nc.scalar.activation(eK[:, st, :], pks[st][:], Act.Exp)

                # ---- V projection (d-half) -> eKV = eK * V ----
                # k-outer like K: wv slices arrive staggered off the gated
                # gpsimd chain, so consume them in arrival order
                for g in range(2):
                    sts = list(range(g * 8, g * 8 + 8))
                    pvs = {
                        st: psum.tile([P, CH], F32, tag="ps", name=f"pv{st}")
                        for st in sts
                    }
                    for k in range(KT):
                        for st in sts:
                            nc.tensor.matmul(
                                pvs[st][:],
                                xs[:, k, st * P : (st + 1) * P],
                                wv_s[:, k, :],
                                start=(k == 0), stop=(k == KT - 1),
                            )
                    for st in sts:
                        nc.vector.tensor_tensor(
                            eKV[:, st, :], eK[:, st, :], pvs[st][:], Alu.mult
                        )

                # ---- Q^T projection (d-half e, ALL t) ----
                for et in range(DHT):
                    esl = slice(et * P, (et + 1) * P)
                    for c in range(TC):
                        tsl = slice(c * CH, (c + 1) * CH)
                        pq = psum.tile([P, CH], F32, tag="ps")
                        for k in range(KT):
                            nc.tensor.matmul(
                                pq[:], wq_s[:, k, esl], xs[:, k, tsl],
                                start=(k == 0), stop=(k == KT - 1),
                            )
                        nc.scalar.activation(
                            sigQT[:, et, tsl], pq[:], Act.Sigmoid,
                            bias=bq_s[:, et : et + 1],
                        )

            # ---- AFT over rotated t-chunks; j = 2,3 (pair-t) first so the
            # pairwise exchanges launch early and hide under compute ----
            b_in = [None, None]
            S_out = [None, None]
            S_sb = [None, None]
            for i in range(2):
                b_in[i] = dram.tile([DH, CH], BF, name=f"bin{i}")
                S_out[i] = dram.tile([2 * DH, CH], BF, name=f"sout{i}")
                S_sb[i] = sgpool.tile(
                    [P, DHT, CH], BF, tag=f"ssb{i}", name=f"ssb{i}"
                )

            # All (dynamic, gpsimd) ew staging DMAs are emitted BEFORE the
            # AllGather triggers: the collective's completion wait blocks
            # the gpsimd FIFO, so anything gpsimd-issued after it would
            # stall behind the exchange.
            for pos, j in enumerate([2, 3, 0, 1]):
                # ew^T chunk for rotated chunk j: t columns
                # (j*512 + p*1024) & 2047
                toff = nc.s_assert_within(
                    (j * CH + par * TH) & (T - 1), 0, T - CH,
                    skip_runtime_assert=True,
                )
                ewc = ewpool.tile([P, ST, CH], BF, tag="ewc")
                nc.gpsimd.dma_start(
                    ewc[:, : ST // 2, :], ew_v[:, : ST // 2, ds(toff, CH)]
                )
                nc.gpsimd.dma_start(
                    ewc[:, ST // 2 :, :], ew_v[:, ST // 2 :, ds(toff, CH)]
                )

                for dti in range(DHT):
                    dsl = slice(dti * P, (dti + 1) * P)
                    pn = psum.tile([P, CH], F32, tag="ps")
                    pd = psum.tile([P, CH], F32, tag="ps")
                    for ss in range(ST):
                        nc.tensor.matmul(
                            pn[:], eKV[:, ss, dsl], ewc[:, ss, :],
                            start=(ss == 0), stop=(ss == ST - 1),
                        )
                    for ss in range(ST):
                        nc.tensor.matmul(
                            pd[:], eK[:, ss, dsl], ewc[:, ss, :],
                            start=(ss == 0), stop=(ss == ST - 1),
                        )
                    rec = tmp.tile([P, CH], F32, tag="rec")
                    nc.vector.reciprocal_approx_fast(rec[:], pd[:])
                    rat = tmp.tile([P, CH], F32, tag="rat")
                    nc.vector.tensor_tensor(rat[:], pn[:], rec[:], Alu.mult)
                    nc.vector.tensor_scalar(
                        rat[:], rat[:], bv_s[:, dti : dti + 1], None, Alu.add
                    )
                    nc.vector.tensor_tensor(
                        oPC[j][:, dti, :], rat[:],
                        sigQT[:, dti, ds(toff, CH)], Alu.mult,
                    )

                if pos < 2:
                    # ship pair-t chunk to the DRAM bounce buffer (sync
                    # engine, so it fires as soon as the chunk is done)
                    nc.sync.dma_start(
                        b_in[pos].rearrange("(o p) t -> p o t", p=P)[:],
                        oPC[j][:],
                    )

            for i in range(2):
                nc.gpsimd.collective_compute(
                    "AllGather",
                    Alu.bypass,
                    replica_groups=PAIRS,
                    ins=[b_in[i].opt()],
                    outs=[S_out[i].opt()],
                )
                # pull the PAIR's contribution rows (their d-half for my
                # own-t chunk i): rank-row select is parity-dependent
                nc.gpsimd.dma_start(
                    S_sb[i][:],
                    S_out[i].rearrange("(o p) t -> p o t", p=P)[
                        :, ds((1 - par) * DHT, DHT), :
                    ],
                )

            # ---- output projection for own-t rows, full d ----
            # contraction order: k 0..3 = my d-half (oPC[0], oPC[1]),
            # k 4..7 = pair d-half (S_sb); woT rows are host-permuted to
            # [my half; pair half].
            wo_s = wopool.tile([P, KT, D], BF, tag="wo", name="wo_s")
            for k in range(KT):
                nc.sync.dma_start(wo_s[:, k, :], wo_v[:, k, :])
            for tt in range(TH // P):
                own = oPC[tt // DHT]        # chunk 0 or 1 (own-t)
                ssb = S_sb[tt // DHT]
                tof = (tt % DHT) * P
                for ec in range(2):
                    esl = slice(ec * CH, (ec + 1) * CH)
                    py = psum.tile([P, CH], F32, tag="ps")
                    for k in range(4):
                        nc.tensor.matmul(
                            py[:], own[:, k, tof : tof + P], wo_s[:, k, esl],
                            start=(k == 0), stop=False,
                        )
                    for k in range(4):
                        nc.tensor.matmul(
                            py[:], ssb[:, k, tof : tof + P],
                            wo_s[:, 4 + k, esl],
                            start=False, stop=(k == 3),
                        )
                    ysb = tmp.tile([P, CH], F32, tag="ysb")
                    nc.vector.tensor_tensor(ysb[:], py[:], bo_s[:, esl], Alu.add)
                    nc.sync.dma_start(y_v[:, tt, esl], ysb[:])

    nc.compile()
    return nc


def _get_nc():
    if "nc" not in _cache:
        _cache["nc"] = _build_nc()
    return _cache["nc"]


def kernel(x, dummy, Wq, bq, Wk, bk, Wv, bv, Wo, bo, wbias):
    import os

    x = np.asarray(x, np.float32)
    Wq = np.asarray(Wq, np.float32)
    Wk = np.asarray(Wk, np.float32)
    Wv = np.asarray(Wv, np.float32)
    Wo = np.asarray(Wo, np.float32)
    bq = np.asarray(bq, np.float32)
    bv = np.asarray(bv, np.float32)
    bo = np.asarray(bo, np.float32)
    wbias = np.asarray(wbias, np.float32)

    wqTf = np.ascontiguousarray(Wq.T)  # [d_in, e_out] fp32
    wkTf = np.ascontiguousarray(Wk.T)
    wvTf = np.ascontiguousarray(Wv.T)
    woTf = np.ascontiguousarray(Wo.T)  # rows = d
    ewTb = np.ascontiguousarray(np.exp(wbias).T).astype(BF16)  # [s, t]
    bob = np.ascontiguousarray(np.broadcast_to(bo, (P, D)))

    in_maps = []
    for c in range(8):
        b, p = c // 2, c % 2
        dlo, dhi = p * DH, (p + 1) * DH
        qlo, qhi = (1 - p) * DH, (2 - p) * DH
        woTp = np.concatenate([woTf[dlo:dhi], woTf[qlo:qhi]], axis=0)
        in_maps.append(
            {
                "xT": np.ascontiguousarray(x[b].T).astype(BF16),
                "wqT": np.ascontiguousarray(wqTf[:, dlo:dhi]).astype(BF16),
                "wkT": np.ascontiguousarray(wkTf[:, dlo:dhi]).astype(BF16),
                "wvT": np.ascontiguousarray(wvTf[:, dlo:dhi]).astype(BF16),
                "woT": np.ascontiguousarray(woTp).astype(BF16),
                "ewT": ewTb,
                "bqc": np.ascontiguousarray(bq[dlo:dhi].reshape(DHT, P).T),
                "bvc": np.ascontiguousarray(bv[dlo:dhi].reshape(DHT, P).T),
                "bob": bob,
            }
        )

    from concourse.bass_utils import run_bass_kernel_spmd

    nc = _get_nc()
    trace = bool(os.environ.get("AFT_TRACE"))
    if not trace:
        # keep run_bass_kernel_spmd off the (environment-dependent) NTFF
        # profiling path unless explicitly requested
        os.environ["BASS_NEVER_TRACE"] = "1"
    res = run_bass_kernel_spmd(
        nc, in_maps, core_ids=list(range(8)), trace=trace
    )
    kernel._last_exec_ns = res.exec_time_ns
    kernel._last_result = res

    out = np.empty((B, T, D), np.float32)
    for c in range(8):
        b, p = c // 2, c % 2
        out[b, p * TH : (p + 1) * TH, :] = res.results[c]["y"]
    return out
